# revision 2
# baseline (speedup 1.0000x reference)
"""Trainium2 Bass kernel for nn_Head (single-head causal attention).

Contract: kernel(**inputs) takes FULL inputs (x [8,2048,1024] f32,
Wk/Wq/Wv [64,1024] f32) and returns the FULL output [8,2048,64] f32.
Data-parallel over batch B=8 across the 8 NeuronCores (one batch row per
core); each core runs an identical single-core program.

Host-side prep (inside kernel(), pure numpy marshaling):
  - xt = x[b].T  (bf16)        -> projections contract over C on partitions
  - wkq = concat([Wk/32, Wq]).T (fold 1/sqrt(C)=1/32 into Wk)
  - wv  = Wv.T

Device kernel (per core), v2 — engineered against the v1 trace:
  * PE HAM warm-up: dummy matmuls + ACT exp-table preload run during the
    input-DMA dead time, so projections and attention run at 2.4 GHz.
  * Projections kq/v interleaved per c-chunk, paced by chunk DMA arrival.
  * kq PSUM->SBUF casts split across DVE (kt) and ACT (qt) so the
    PE->attention boundary has no multi-us engine stall (v1 re-throttled
    the PE clock to 1.2 GHz here and lost ~20us).
  * v-transposes write PSUM tiles that reuse the freed kq banks; vaug
    copies are 4x[128,4*64] instead of 16x[128,64].
  * Attention: ST (scores^T) tiles fp32 in PSUM, one exp per 1024-wide
    tile on ACT (the pacing engine, ~21us), diagonal causal mask via
    gpsimd affine_select (keeps DVE light), PV accumulates [65,512]
    PSUM banks with a ones-column denominator row.
  * Epilogue per 512-column bank as soon as its accumulation stops
    (i=4j+3): copy, reciprocal of the denominator row, PE transpose,
    scale, and per-128-row output DMA — all overlapped under the
    remaining attention instead of serialized at the end.
"""

import sys

if "/opt/trn_rl_repo" not in sys.path:
    sys.path.insert(0, "/opt/trn_rl_repo")

import numpy as np

B = 8
T = 2048
C = 1024
H = 64
P = 128
CB = C // P        # 8 contraction chunks
TJ = T // 512      # 4 column chunks of 512
NT = T // P        # 16 s-tiles
N_CORES = 8

_NC_CACHE = {}


def _build_nc():
    import concourse.bass as bass
    import concourse.mybir as mybir
    import concourse.tile as tile
    from concourse.bass import ts
    from concourse.masks import make_identity

    fp32 = mybir.dt.float32
    bf16 = mybir.dt.bfloat16
    EXP = mybir.ActivationFunctionType.Exp
    COPY = mybir.ActivationFunctionType.Copy

    nc = bass.Bass(target_bir_lowering=False, debug=False)
    xt_d = nc.declare_dram_parameter("xt", [C, T], bf16, isOutput=False)
    wkq_d = nc.declare_dram_parameter("wkq", [C, P], bf16, isOutput=False)
    wv_d = nc.declare_dram_parameter("wv", [C, H], bf16, isOutput=False)
    out_d = nc.declare_dram_parameter("out", [T, H], fp32, isOutput=True)

    from contextlib import ExitStack

    with tile.TileContext(nc) as tc, ExitStack() as stk:
        pers = stk.enter_context(tc.tile_pool(name="pers", bufs=1))
        xt_sb = pers.tile([P, CB, T], bf16, tag="xt_sb", name="xt_sb")
        wkq_sb = pers.tile([P, CB, P], bf16, tag="wkq_sb", name="wkq_sb")
        wv_sb = pers.tile([P, CB, H], bf16, tag="wv_sb", name="wv_sb")
        kt_sb = pers.tile([H, T], bf16, tag="kt_sb", name="kt_sb")
        qt_sb = pers.tile([H, T], bf16, tag="qt_sb", name="qt_sb")
        vt_sb = pers.tile([H, T], bf16, tag="vt_sb", name="vt_sb")
        vaug_sb = pers.tile([P, NT, H + 1], bf16, tag="vaug_sb", name="vaug_sb")
        ot_sb = pers.tile([H + 1, T], fp32, tag="ot_sb", name="ot_sb")
        o_sb = pers.tile([P, NT, H], fp32, tag="o_sb", name="o_sb")
        ident = pers.tile([P, P], fp32, tag="ident", name="ident")
        identb = pers.tile([H, H], bf16, tag="identb", name="identb")
        dummy_sb = pers.tile([H, 256], bf16, tag="dummy_sb", name="dummy_sb")
        tl_sb = pers.tile([1, 8], fp32, tag="tl_sb", name="tl_sb")

        # ---- input DMAs first (sync HWDGE queue, strict order: weights,
        # then x chunks 0..7 so arrival order matches consumption order) ----
        nc.sync.dma_start(wkq_sb[:], wkq_d.rearrange("(o p) m -> p o m", p=P))
        nc.sync.dma_start(wv_sb[:], wv_d.rearrange("(o p) m -> p o m", p=P))
        for cb in range(CB):
            nc.sync.dma_start(xt_sb[:, cb, :], xt_d[cb * P : (cb + 1) * P, :])

        # ---- setup constants; preload the exp table on ACT while DMA runs ----
        make_identity(nc, ident[:])
        make_identity(nc, identb[:])
        nc.gpsimd.memset(dummy_sb[:], 0.0)
        nc.gpsimd.memset(tl_sb[:], 0.0)
        nc.any.memset(vaug_sb[:, :, H], 1.0)
        nc.scalar.activation(tl_sb[:], tl_sb[:], EXP)

        # ---- HAM warm-up: ~3.4us of dummy matmuls while the first x chunk
        # is still in flight, so real matmuls start at 2.4 GHz ----
        with tc.tile_pool(name="warm", bufs=1, space="PSUM") as wp:
            wps = wp.tile([H, 256], fp32, tag="w", name="warm")
            for _ in range(16):
                nc.tensor.matmul(wps, identb[:], dummy_sb[:], start=True, stop=True)

        # ---- projections: kqT [128, T] and vT [64, T], interleaved per chunk ----
        with tc.tile_pool(name="pp", bufs=4, space="PSUM") as pp:
            kq_ps = [pp.tile([P, 512], fp32, tag="kq", name=f"kq{j}") for j in range(TJ)]
            v_ps = [pp.tile([H, 512], fp32, tag="v", name=f"v{j}") for j in range(TJ)]
            for cb in range(CB):
                for j in range(TJ):
                    nc.tensor.matmul(
                        kq_ps[j], wkq_sb[:, cb, :], xt_sb[:, cb, ts(j, 512)],
                        start=(cb == 0), stop=(cb == CB - 1),
                    )
                for j in range(TJ):
                    nc.tensor.matmul(
                        v_ps[j], wv_sb[:, cb, :], xt_sb[:, cb, ts(j, 512)],
                        start=(cb == 0), stop=(cb == CB - 1),
                    )

            # PSUM -> SBUF casts: kt+vt on DVE, qt on ACT (parallel engines)
            for j in range(TJ):
                nc.vector.tensor_copy(kt_sb[:, ts(j, 512)], kq_ps[j][0:H, :])
                nc.scalar.activation(qt_sb[:, ts(j, 512)], kq_ps[j][H:P, :], COPY)
                nc.vector.tensor_copy(vt_sb[:, ts(j, 512)], v_ps[j][:, :])

            # v natural [s, d]: PE transposes in groups of 4 into one PSUM
            # tile (reusing freed kq banks), one wide DVE copy per group
            for g in range(4):
                vg = pp.tile([P, 4, H], bf16, tag="kq", name=f"vg{g}")
                for t in range(4):
                    i = 4 * g + t
                    nc.tensor.transpose(vg[:, t, :], vt_sb[:, ts(i, P)], identb[:])
                nc.vector.tensor_copy(vaug_sb[:, 4 * g : 4 * g + 4, 0:H], vg)

        # ---- attention: ST tiles 1024 wide, one exp per tile, PV lags ST
        # by one s-tile; per-bank epilogue as soon as accumulation stops ----
        out_r = out_d.rearrange("(i p) d -> p i d", p=P)
        with (
            tc.tile_pool(name="stp", bufs=2, space="PSUM") as stp,
            tc.tile_pool(name="otp", bufs=4, space="PSUM") as otp,
            tc.tile_pool(name="ptp", bufs=6) as ptp,
        ):
            ot_ps = [otp.tile([H + 1, 512], fp32, tag="ot", name=f"ot{j}") for j in range(TJ)]

            def emit_st(i):
                j0 = i // 4
                pts = {}
                for jj2 in range(i // 8, 2):
                    st = stp.tile([P, 1024], fp32, tag="st", name=f"st{i}_{jj2}")
                    pt = ptp.tile([P, 1024], bf16, tag="pt", name=f"pt{i}_{jj2}")
                    estart = None
                    for hh in range(2):
                        j = 2 * jj2 + hh
                        if j < j0:
                            continue
                        o = max(0, 128 * i - 512 * j)
                        lo = 512 * hh + o
                        nc.tensor.matmul(
                            st[:, lo : 512 * (hh + 1)], qt_sb[:, ts(i, P)],
                            kt_sb[:, 512 * j + o : 512 * (j + 1)],
                            start=True, stop=True,
                        )
                        if estart is None:
                            estart = lo
                    nc.scalar.activation(pt[:, estart:1024], st[:, estart:1024], EXP)
                    if jj2 == i // 8:
                        # causal mask of the diagonal 128x128 block:
                        # keep pt[s, t] where t - s >= 0, else 0
                        dlo = 128 * (i % 8)
                        nc.gpsimd.affine_select(
                            out=pt[:, dlo : dlo + P],
                            in_=pt[:, dlo : dlo + P],
                            pattern=[[1, P]],
                            compare_op=mybir.AluOpType.is_ge,
                            fill=0.0,
                            base=0,
                            channel_multiplier=-1,
                        )
                    pts[jj2] = pt
                return pts

            def emit_pv(i, pts):
                j0 = i // 4
                for j in range(j0, TJ):
                    o = max(0, 128 * i - 512 * j)
                    pt = pts[j // 2]
                    lo = 512 * (j % 2) + o
                    nc.tensor.matmul(
                        ot_ps[j][:, o:512], vaug_sb[:, i, :],
                        pt[:, lo : 512 * (j % 2) + 512],
                        start=(i == 0), stop=(i == 4 * j + 3),
                    )

            def epilogue(j):
                # bank j finished accumulating at i=4j+3: copy out, invert
                # the denominator row once, then per-128-row transpose,
                # scale, and DMA — all overlapped under later attention
                nc.vector.tensor_copy(ot_sb[:, ts(j, 512)], ot_ps[j])
                nc.vector.reciprocal(
                    ot_sb[H : H + 1, ts(j, 512)], ot_sb[H : H + 1, ts(j, 512)]
                )
                for ii in range(4 * j, 4 * j + 4):
                    ops = stp.tile([P, H + 1], fp32, tag="st", name=f"or{ii}")
                    nc.tensor.transpose(
                        ops, ot_sb[:, ts(ii, P)], ident[0 : H + 1, 0 : H + 1]
                    )
                    nc.vector.tensor_scalar_mul(
                        o_sb[:, ii, :], ops[:, 0:H], ops[:, H : H + 1]
                    )
                    nc.sync.dma_start(out_r[:, ii, :], o_sb[:, ii, :])

            prev = None
            for i in range(NT):
                pts = emit_st(i)
                if prev is not None:
                    emit_pv(prev[0], prev[1])
                    if prev[0] % 4 == 3:
                        epilogue(prev[0] // 4)
                prev = (i, pts)
            emit_pv(prev[0], prev[1])
            epilogue(3)

    return nc


def _split_multiwaits(nc):
    """Walrus codegen only supports one sync-wait command per instruction;
    hoist extra waits onto NoOps inserted just before (same engine queue,
    identical semantics since engines execute their queue in order)."""
    import concourse.mybir as mybir

    n = 0
    for fn in nc.m.functions:
        for block in fn.blocks:
            new_insts = []
            for inst in block.instructions:
                si = inst.sync_info
                if si is not None and si.on_wait and len(si.on_wait) > 1:
                    waits = list(si.on_wait)
                    for w in waits[:-1]:
                        n += 1
                        new_insts.append(
                            mybir.InstNoOp(
                                name=f"WH-{n}", engine=inst.engine, ins=[], outs=[],
                                sync_info=mybir.SyncInfo(on_wait=[w], on_update=[]),
                            )
                        )
                    si.on_wait = waits[-1:]
                new_insts.append(inst)
            block.instructions = new_insts
    return nc


def _get_nc():
    if "nc" not in _NC_CACHE:
        _NC_CACHE["nc"] = _split_multiwaits(_build_nc())
    return _NC_CACHE["nc"]


def _make_in_maps(x, Wk, Wq, Wv):
    import ml_dtypes

    bf16 = ml_dtypes.bfloat16
    scale = 1.0 / np.sqrt(np.float32(C))
    wkq = np.ascontiguousarray(
        np.concatenate([Wk * scale, Wq], axis=0).T.astype(bf16)
    )  # [C, 128]
    wv = np.ascontiguousarray(Wv.T.astype(bf16))  # [C, 64]
    in_maps = []
    for b in range(B):
        xt = np.ascontiguousarray(x[b].T.astype(bf16))  # [C, T]
        in_maps.append({"xt": xt, "wkq": wkq, "wv": wv})
    return in_maps


def run(x, Wk, Wq, Wv, trace=False):
    from concourse.bass_utils import run_bass_kernel_spmd

    nc = _get_nc()
    in_maps = _make_in_maps(x, Wk, Wq, Wv)
    res = run_bass_kernel_spmd(nc, in_maps, core_ids=list(range(N_CORES)), trace=trace)
    out = np.stack([np.asarray(res.results[b]["out"]) for b in range(B)], axis=0)
    return out.astype(np.float32), res


def kernel(x, Wk, Wq, Wv):
    out, _ = run(x, Wk, Wq, Wv, trace=False)
    return out
